# revision 1
# baseline (speedup 1.0000x reference)
"""Co-attention kernel for Trainium2, 8-core data-parallel over batch.

reference math (per batch):
  a  = q @ d.T                      [Lq, Ld]
  aq = softmax_q(mask_q(a))         (softmax over dim q)
  ad = softmax_d(mask_d(a.T))       (softmax over dim d)
  sd = q.T @ aq                     [H, Ld]
  sq = d.T @ ad                     [H, Lq]
  cd = sq @ aq                      [H, Ld]
  returns (cd.T, sq.T, sd.T)        ([Ld,H], [Lq,H], [Ld,H])

Distribution: pure data parallel — batch 32 is split 4-per-core across the
8 NeuronCores; each core runs an identical Bass/Tile program.

On-chip strategy (per batch, per core; all matmuls in f32r = full PE rate):
  QT [h,q], DT [h,d]: loaded directly (host supplies q/d transposed copies)
  ATm [d,q] = DT.T @ QT (+maskq fused into the PSUM->SBUF copy on DVE)
  A   [q,d] = PE-transpose(ATm) with +maskd fused into copy-out
              (the -maskq offset is constant per A-row, cancels in softmax-d)
  softmax-q on ATm rows: rowmax (DVE), exp with fused bias=-max and fused
    row-sum (ACT, f32r out), reciprocal -> EqT [d,q]; PE-transpose -> Eq [q,d]
  softmax-d on A rows: same -> EdT [q,d]; PE-transpose -> EdL [d,q]
  (exp tiles stay UNNORMALIZED; the 1/sum is per-partition in each output's
   layout and is folded into the output PSUM->SBUF copies on ACT)
  sdT [d,h] = Eq.T @ Qn  * inv_q  -> out sd.T
  sqT [q,h] = EdL.T @ Dn * inv_d  -> out sq.T
  cdT [d,h] = Eq.T @ sqT * inv_q  -> out cd.T
"""

import hashlib
import os
import shutil
import tempfile
from pathlib import Path

import numpy as np

B, L, H = 32, 1024, 1024  # Lq == Ld == H == 1024
NCORES = 8
BPC = B // NCORES  # batches per core
NT = L // 128      # 8 row-tiles per matrix
# Additive mask constant. Chosen small enough that f32 keeps ~1e-3 absolute
# precision on masked logits (ulp(1e4) ~ 9.8e-4) — the constant cancels
# exactly in softmax-d's per-row shift and underflows exp to 0 in softmax-q
# (exp(x) == 0 in f32 for x < -103).
NEG = -10000.0

_NEFF_CACHE = os.environ.get(
    "NEFF_CACHE_DIR", os.path.join(tempfile.gettempdir(), "neff_cache")
)


def _install_neff_cache():
    import concourse.bass2jax as b2j

    orig = b2j.compile_bir_kernel
    if getattr(b2j, "_neff_cache_installed", False):
        return
    os.makedirs(_NEFF_CACHE, exist_ok=True)

    def cached(bir_json, tmpdir, neff_name="file.neff"):
        if isinstance(bir_json, str):
            bir_json = bir_json.encode()
        key = hashlib.sha256(bir_json).hexdigest()
        hit = Path(_NEFF_CACHE) / f"{key}.neff"
        out = Path(tmpdir) / neff_name
        if hit.exists():
            shutil.copyfile(hit, out)
            return str(out)
        res = orig(bir_json, tmpdir, neff_name)
        try:
            shutil.copyfile(res, hit)
        except OSError:
            pass
        return res

    b2j.compile_bir_kernel = cached
    b2j._neff_cache_installed = True


def build_module(bpc=BPC, reps=1):
    """Build + compile the per-core Bass module. Returns the Bacc object."""
    import concourse.bacc as bacc
    import concourse.bass as bass
    import concourse.tile as tile
    from concourse import mybir
    from concourse.masks import make_identity

    f32 = mybir.dt.float32
    f32r = mybir.dt.float32r
    i32 = mybir.dt.int32

    nc = bacc.Bacc("TRN2", target_bir_lowering=False, debug=False)

    host_t = os.environ.get("COATT_HOST_T", "1") == "1"
    q_d = nc.dram_tensor("q", [bpc, L, H], f32r, kind="ExternalInput")
    d_d = nc.dram_tensor("d", [bpc, L, H], f32r, kind="ExternalInput")
    qt_d = dt_d = None
    if host_t:
        qt_d = nc.dram_tensor("qt", [bpc, H, L], f32r, kind="ExternalInput")
        dt_d = nc.dram_tensor("dt", [bpc, H, L], f32r, kind="ExternalInput")
    qlen_d = nc.dram_tensor("qlen", [bpc], f32, kind="ExternalInput")
    dlen_d = nc.dram_tensor("dlen", [bpc], f32, kind="ExternalInput")
    cd_d = nc.dram_tensor("cd", [bpc, L, H], f32, kind="ExternalOutput")
    sq_d = nc.dram_tensor("sq", [bpc, L, H], f32, kind="ExternalOutput")
    sd_d = nc.dram_tensor("sd", [bpc, L, H], f32, kind="ExternalOutput")

    with tile.TileContext(nc) as tc:
        _build_body(nc, tc, bass, mybir, make_identity,
                    q_d, d_d, qlen_d, dlen_d, cd_d, sq_d, sd_d, bpc, reps,
                    qt_d=qt_d, dt_d=dt_d)

    nc.compile()
    return nc


def _build_body(nc, tc, bass, mybir, make_identity,
                q_d, d_d, qlen_d, dlen_d, cd_d, sq_d, sd_d, bpc, reps,
                qt_d=None, dt_d=None):
    from contextlib import ExitStack

    f32 = mybir.dt.float32
    f32r = mybir.dt.float32r
    i32 = mybir.dt.int32

    big_bufs = int(os.environ.get("COATT_BIGBUFS", "42"))
    pmm_bufs = int(os.environ.get("COATT_PMM", "5"))
    ptr_bufs = int(os.environ.get("COATT_PTR", "3"))
    with ExitStack() as ctx:
        const = ctx.enter_context(tc.tile_pool(name="const", bufs=1))
        big = ctx.enter_context(tc.tile_pool(name="big", bufs=big_bufs))
        stage = ctx.enter_context(tc.tile_pool(name="stage", bufs=4))
        maskp = ctx.enter_context(tc.tile_pool(name="maskp", bufs=2))
        small = ctx.enter_context(tc.tile_pool(name="small", bufs=24))
        pmm = ctx.enter_context(
            tc.tile_pool(name="pmm", bufs=pmm_bufs, space="PSUM"))
        ptr = ctx.enter_context(
            tc.tile_pool(name="ptr", bufs=ptr_bufs, space="PSUM"))

        # --- constants -------------------------------------------------
        ident = const.tile([128, 128], f32)
        make_identity(nc, ident)
        ident_r = const.tile([128, 128], f32r)
        nc.vector.tensor_copy(ident_r, ident)
        iota_i = const.tile([128, L], i32)
        nc.gpsimd.iota(iota_i, pattern=[[1, L]], base=0, channel_multiplier=0)
        iota_f = const.tile([128, L], f32)
        nc.vector.tensor_copy(iota_f, iota_i)

        def mat(name):
            # allocate one [1024, 1024] matrix as 8 tiles [128, 1024]
            return [big.tile([128, L], f32, name=f"{name}_{r}", tag="mat")
                    for r in range(NT)]

        def mat_r(name):
            return [big.tile([128, L], f32r, name=f"{name}_{r}", tag="mat")
                    for r in range(NT)]

        def load_mat(dst, dram, b):
            for r in range(NT):
                nc.sync.dma_start(
                    out=dst[r], in_=dram.ap()[b, 128 * r:128 * (r + 1), :])

        def pe_transpose(src, dst_dtype_r, name, fuse_add=None, out_dt=None):
            """dst = src.T (8x8 grid of 128x128 PE transposes).

            src: list of 8 tiles [128, L]; returns new mat tiles.
            fuse_add: optional [128, L] mask tile added during copy-out (DVE).
            dst_dtype_r: True -> dst tiles f32r (copy-out converts).
            """
            dst = mat_r(name) if dst_dtype_r else mat(name)
            src_r = src[0].dtype == f32r
            idn = ident_r if src_r else ident
            pdt = f32r if src_r else f32
            for r2 in range(NT):
                for cg in range(2):  # two 512-wide column groups
                    pst = ptr.tile([128, 512], pdt, name=f"pst_{name}", tag="pst")
                    for cc in range(4):
                        c = 4 * cg + cc
                        nc.tensor.transpose(
                            pst[:, 128 * cc:128 * (cc + 1)],
                            src[c][:, 128 * r2:128 * (r2 + 1)],
                            idn)
                    out_sl = dst[r2][:, 512 * cg:512 * (cg + 1)]
                    if fuse_add is not None:
                        nc.vector.tensor_add(
                            out_sl, pst, fuse_add[:, 512 * cg:512 * (cg + 1)])
                    else:
                        nc.scalar.copy(out=out_sl, in_=pst)
            return dst

        def emit_mm(lhsT, rhs, consume, name):
            """out[m,n] = sum_k lhsT[k][:,m-block] . rhs[k][:,n-strip].

            lhsT: 8 k-tiles [128, L(m)]; rhs: 8 k-tiles [128, L(n)].
            consume(r, ns, psum_ap) for each (m-tile r, 512-strip ns).
            """
            for r in range(NT):
                for ns in range(2):
                    ps = pmm.tile([128, 512], f32, name=f"ps_{name}", tag="ps")
                    for k in range(NT):
                        nc.tensor.matmul(
                            ps,
                            lhsT[k][:, 128 * r:128 * (r + 1)],
                            rhs[k][:, 512 * ns:512 * (ns + 1)],
                            start=(k == 0), stop=(k == NT - 1))
                    consume(r, ns, ps)

        def softmax_rows(src, name):
            """Masked already; UNNORMALIZED exp along free dim of each tile.

            exp (ACT, f32r out, fused row-sum) -> reciprocal. Normalization is
            deferred to the downstream PSUM->SBUF output copies, where the inv
            is a per-partition scale in the final output layouts — this keeps
            exp -> transpose off the recip/scale dependency chain.
            """
            out = mat_r(name)
            invs = []
            for r in range(NT):
                mx = small.tile([128, 1], f32, name=f"mx_{name}", tag="mx")
                nc.vector.reduce_max(mx, src[r], axis=mybir.AxisListType.X)
                nmx = small.tile([128, 1], f32, name=f"nmx_{name}", tag="nmx")
                nc.vector.tensor_scalar_mul(nmx, mx, -1.0)
                sm = small.tile([128, 1], f32, name=f"sm_{name}", tag="sm")
                nc.scalar.activation(
                    out=out[r], in_=src[r],
                    func=mybir.ActivationFunctionType.Exp,
                    bias=nmx, scale=1.0, accum_out=sm)
                inv = small.tile([128, 1], f32, name=f"inv_{name}", tag="inv")
                nc.vector.reciprocal(inv, sm)
                invs.append(inv)
            return out, invs

        def bcast_len(dram, b, name):
            t = small.tile([128, 1], f32, name=name, tag=name)
            src = bass.AP(tensor=dram, offset=b, ap=[[0, 128], [1, 1]])
            nc.sync.dma_start(out=t, in_=src)
            return t

        for _rep in range(reps):
            for b in range(bpc):
                # --- load + masks -----------------------------------------
                qlen = bcast_len(qlen_d, b, "qlen_t")
                dlen = bcast_len(dlen_d, b, "dlen_t")
                maskq = maskp.tile([128, L], f32, name="maskq", tag="mk")
                nc.vector.tensor_scalar(
                    out=maskq, in0=iota_f, scalar1=qlen, scalar2=NEG,
                    op0=mybir.AluOpType.is_ge, op1=mybir.AluOpType.mult)
                maskd = maskp.tile([128, L], f32, name="maskd", tag="mk")
                nc.vector.tensor_scalar(
                    out=maskd, in0=iota_f, scalar1=dlen, scalar2=NEG,
                    op0=mybir.AluOpType.is_ge, op1=mybir.AluOpType.mult)

                if qt_d is not None:
                    QT = mat_r("QT")
                    load_mat(QT, qt_d, b)
                    DT = mat_r("DT")
                    load_mat(DT, dt_d, b)
                else:
                    Qn = mat_r("Qn")
                    load_mat(Qn, q_d, b)
                    QT = pe_transpose(Qn, True, "QT")
                    Dn = mat_r("Dn")
                    load_mat(Dn, d_d, b)
                    DT = pe_transpose(Dn, True, "DT")

                # --- ATm = DT.T @ QT + maskq  ([d, q]) --------------------
                ATm = mat("ATm")

                def at_consume(r, ns, ps):
                    sl = slice(512 * ns, 512 * (ns + 1))
                    nc.vector.tensor_add(ATm[r][:, sl], ps, maskq[:, sl])

                emit_mm(DT, QT, at_consume, "at")

                # --- A = ATm.T + maskd  ([q, d]) --------------------------
                # The -maskq offset carried in masked q-rows is a per-row
                # constant in A's layout, so it cancels exactly in
                # softmax-d's max-shift; NEG is small enough that f32 keeps
                # the logits' precision under the offset.
                A = pe_transpose(ATm, False, "A", fuse_add=maskd)

                # --- softmax over q (on ATm rows) -------------------------
                EqT, inv_q = softmax_rows(ATm, "EqT")  # [d, q] ~ aq.T (f32r)
                Eq = pe_transpose(EqT, True, "Eq")     # [q, d] ~ aq (f32r)

                # --- softmax over d (on A rows) ---------------------------
                EdT, inv_d = softmax_rows(A, "EdT")    # [q, d] ~ ad.T (f32r)
                EdL = pe_transpose(EdT, True, "EdL")   # [d, q] ~ ad (f32r)

                # --- sdT = Eq.T @ Qn  ([d, h]) ----------------------------
                Qn2 = mat_r("Qn2")
                load_mat(Qn2, q_d, b)
                sd_stage = {}

                def sd_consume(r, ns, ps):
                    if r not in sd_stage:
                        sd_stage[r] = stage.tile(
                            [128, L], f32, name="sd_st", tag="st")
                    st = sd_stage[r]
                    nc.scalar.activation(
                        out=st[:, 512 * ns:512 * (ns + 1)], in_=ps,
                        func=mybir.ActivationFunctionType.Copy,
                        scale=inv_q[r])
                    if ns == 1:
                        nc.sync.dma_start(
                            out=sd_d.ap()[b, 128 * r:128 * (r + 1), :], in_=st)

                emit_mm(Eq, Qn2, sd_consume, "sd")

                # --- sqT = EdL.T @ Dn  ([q, h]) ---------------------------
                Dn2 = mat_r("Dn2")
                load_mat(Dn2, d_d, b)
                sqT = mat_r("sqT")

                def sq_consume(r, ns, ps):
                    nc.scalar.activation(
                        out=sqT[r][:, 512 * ns:512 * (ns + 1)], in_=ps,
                        func=mybir.ActivationFunctionType.Copy,
                        scale=inv_d[r])
                    if ns == 1:
                        nc.sync.dma_start(
                            out=sq_d.ap()[b, 128 * r:128 * (r + 1), :],
                            in_=sqT[r].bitcast(mybir.dt.float32))

                emit_mm(EdL, Dn2, sq_consume, "sq")

                # --- cdT = Eq.T @ sqT  ([d, h]) ---------------------------
                cd_stage = {}

                def cd_consume(r, ns, ps):
                    if r not in cd_stage:
                        cd_stage[r] = stage.tile(
                            [128, L], f32, name="cd_st", tag="st")
                    st = cd_stage[r]
                    nc.scalar.activation(
                        out=st[:, 512 * ns:512 * (ns + 1)], in_=ps,
                        func=mybir.ActivationFunctionType.Copy,
                        scale=inv_q[r])
                    if ns == 1:
                        nc.sync.dma_start(
                            out=cd_d.ap()[b, 128 * r:128 * (r + 1), :], in_=st)

                emit_mm(Eq, sqT, cd_consume, "cd")


_MODULE = None


def _get_module():
    global _MODULE
    if _MODULE is None:
        _install_neff_cache()
        _MODULE = build_module()
    return _MODULE


def build_in_maps(q, d, q_len, d_len):
    q = np.ascontiguousarray(q, dtype=np.float32)
    d = np.ascontiguousarray(d, dtype=np.float32)
    qlen_f = np.ascontiguousarray(q_len).astype(np.float32)
    dlen_f = np.ascontiguousarray(d_len).astype(np.float32)
    host_t = os.environ.get("COATT_HOST_T", "1") == "1"
    if host_t:
        qt = np.ascontiguousarray(q.transpose(0, 2, 1))
        dt = np.ascontiguousarray(d.transpose(0, 2, 1))

    in_maps = []
    for c in range(NCORES):
        s = slice(c * BPC, (c + 1) * BPC)
        m = {"q": q[s], "d": d[s], "qlen": qlen_f[s], "dlen": dlen_f[s]}
        if host_t:
            m["qt"] = qt[s]
            m["dt"] = dt[s]
        in_maps.append(m)
    return in_maps


def kernel(q, d, q_len, d_len):
    from concourse.bass_utils import run_bass_kernel_spmd

    nc = _get_module()
    in_maps = build_in_maps(q, d, q_len, d_len)

    res = run_bass_kernel_spmd(nc, in_maps, core_ids=list(range(NCORES)))
    cd = np.concatenate([res.results[c]["cd"] for c in range(NCORES)], axis=0)
    sq = np.concatenate([res.results[c]["sq"] for c in range(NCORES)], axis=0)
    sd = np.concatenate([res.results[c]["sd"] for c in range(NCORES)], axis=0)
    return cd, sq, sd

